# revision 5
# baseline (speedup 1.0000x reference)
"""Trainium2 Bass kernel for nn_BlurTensor: gaussian_filter(sigma=k_size) over
ALL axes of x (B=32, C=3, H=512, W=512) with 'symmetric' boundary.

Decomposition: the blur is the linear operator A0 (x) A1 (x) A2 (x) A3 applied
as mode products (one blur matrix per axis, built on host from k_size).
B and C fold into a single 96x96 Kronecker-product mixing matrix (96 <= 128
partitions), so the device does three matmul passes: H, W (banded), BC.

Sharding: H axis split into 8 x 64-row output slabs; each core receives a
104-row input slab (radius-20 halo), which makes all passes core-local.

Device compute dtype is float16 for all streamed data and blur matrices
(PSUM accumulation stays fp32): halves HBM traffic and weight-load time.
Measured end-to-end max rel err ~7e-4 (tolerance 2e-2).

Device pipeline per core (all intermediates SBUF-resident):
  pass H : out[w, (bc,h)] via lhsT = data slab [h'=104, w-chunk 128],
           rhs = A2_local^T [104, 64]  -> X1 [P:w(4x128), F:(wc,bc,h)]
  pass W : banded 512x512 matrix, 4 chunked matmuls accumulating into one
           PSUM bank per h (start=True on first clears has_written; the
           overlapping band writes then accumulate-or-overwrite per element)
  pass BC: Kronecker matrix as stationary weights, rhs = X2h [96, 512]
"""

import numpy as np

import concourse.bass as bass
import concourse.bacc as bacc
import concourse.mybir as mybir
from concourse.tile import TileContext
from concourse.bass_utils import run_bass_kernel_spmd

TRUNCATE = 4.0
N_CORES = 8
F32 = mybir.dt.float32
DEV_DT = mybir.dt.float16
DEV_NP = np.float16

# problem geometry (hardcoded per contest contract)
B, C, H, W = 32, 3, 512, 512
BC = B * C
HS = H // N_CORES          # 64 output rows per core
NJ = W // 128              # w chunks


def _gauss_kernel(sigma):
    # matches scipy/jax _gaussian_kernel1d in float32, like the reference
    radius = int(TRUNCATE * sigma + 0.5)
    x = np.arange(-radius, radius + 1, dtype=np.float32)
    w = np.exp(-0.5 * (x / sigma) ** 2).astype(np.float32)
    w = (w / w.sum(dtype=np.float32)).astype(np.float32)
    return w.astype(np.float64), radius


def _blur_matrix(L, w, radius):
    """(L, L) float64 operator: blur(v) = A @ v with symmetric padding."""
    I = np.eye(L, dtype=np.float64)
    Ipad = np.pad(I, ((radius, radius), (0, 0)), mode="symmetric")
    A = np.empty((L, L), dtype=np.float64)
    for i in range(L):
        A[i, :] = w @ Ipad[i : i + 2 * radius + 1, :]
    return A


def build_program(hin, radius, reps=1):
    """SPMD Bass program (identical on all cores). `reps` repeats the whole
    blur body inside one NEFF (used by test.py slope timing)."""
    hs = HS
    assert BC <= 128 and hin <= 128 and W % 128 == 0

    nc = bacc.Bacc("TRN2")
    xs = nc.dram_tensor("xs", [B, C, hin, W], DEV_DT, kind="ExternalInput")
    a2lt = nc.dram_tensor("a2lt", [hin, hs], DEV_DT, kind="ExternalInput")
    a3t = nc.dram_tensor("a3t", [NJ, 128, W], DEV_DT, kind="ExternalInput")
    mbct = nc.dram_tensor("mbct", [BC, BC], DEV_DT, kind="ExternalInput")
    out = nc.dram_tensor("out", [B, C, hs, W], DEV_DT, kind="ExternalOutput")

    xs_r = xs[:].rearrange("b c h w -> h (b c) w")
    out_r = out[:].rearrange("b c h w -> (b c) h w")

    GRP = 8     # bc per matmul/psum group
    LDG = 16    # bc per input DMA (2 matmul groups -> ~1.7MB per transfer)
    n_ld = BC // LDG
    HB = 8      # h rows per output stage/DMA

    def band(j):
        return max(0, 128 * j - radius), min(W, 128 * j + 128 + radius)

    with TileContext(nc) as tc:
        with (
            tc.tile_pool(name="const", bufs=1) as cpool,
            tc.tile_pool(name="x1p", bufs=1) as x1pool,
        ):
            t_a2lt = cpool.tile([hin, hs], DEV_DT)
            nc.sync.dma_start(out=t_a2lt[:], in_=a2lt[:])
            t_a3t = cpool.tile([128, NJ, W], DEV_DT)
            nc.sync.dma_start(out=t_a3t[:], in_=a3t[:].rearrange("j p n -> p j n"))
            t_mbct = cpool.tile([BC, BC], DEV_DT)
            nc.sync.dma_start(out=t_mbct[:], in_=mbct[:])

            t_x1 = x1pool.tile([128, NJ, BC, hs], DEV_DT)

            for _rep in range(reps):
                # ---------------- pass H (DMA-streamed groups) ----------------
                with (
                    tc.tile_pool(name="ld", bufs=3) as ldpool,
                    tc.tile_pool(name="psh", bufs=4, space="PSUM") as ph,
                ):
                    for ld in range(n_ld):
                        xt = ldpool.tile([hin, LDG, W], DEV_DT, tag="xt")
                        nc.sync.dma_start(
                            out=xt[:], in_=xs_r[:, ld * LDG : (ld + 1) * LDG, :]
                        )
                        # one 2-bank PSUM tile per (ld, j): 16 bc x 64 h,
                        # copied out in a single [128, 1024] op
                        for j in range(NJ):
                            ps = ph.tile([128, LDG * hs], F32, tag="ph")
                            for i in range(LDG):
                                nc.tensor.matmul(
                                    ps[:, i * hs : (i + 1) * hs],
                                    lhsT=xt[:, i, 128 * j : 128 * (j + 1)],
                                    rhs=t_a2lt[:],
                                    start=True,
                                    stop=True,
                                )
                            dst = t_x1[:, j, ld * LDG : (ld + 1) * LDG, :]
                            if (ld * NJ + j) % 2 == 0:
                                nc.vector.tensor_copy(dst, ps[:])
                            else:
                                nc.scalar.copy(dst, ps[:])

                # ------------- pass W + BC (fused, skewed per h) --------------
                with (
                    tc.tile_pool(name="x2p", bufs=4) as x2pool,
                    tc.tile_pool(name="stg", bufs=2) as stpool,
                    tc.tile_pool(name="psw", bufs=2, space="PSUM") as pw,
                    tc.tile_pool(name="psb", bufs=2, space="PSUM") as pb,
                ):
                    # h rows processed in pairs; each pair uses one 2-bank
                    # PSUM tile and a single wide copy per stage
                    SKEW = 1
                    nhp = hs // 2
                    x2_tiles = {}
                    stage = [None]

                    def emit_w(hp):
                        ps = pw.tile([BC, 2, W], F32, tag="w")
                        for q in range(2):
                            h = 2 * hp + q
                            for j in range(NJ):
                                lo, hi = band(j)
                                nc.tensor.matmul(
                                    ps[:, q, lo:hi],
                                    lhsT=t_x1[:, j, :, h],
                                    rhs=t_a3t[:, j, lo:hi],
                                    start=(j == 0),
                                    stop=(j == NJ - 1),
                                )
                        x2 = x2pool.tile([BC, 2, W], DEV_DT, tag="x2")
                        if hp % 2 == 0:
                            nc.vector.tensor_copy(x2[:], ps[:])
                        else:
                            nc.scalar.copy(x2[:], ps[:])
                        x2_tiles[hp] = x2

                    def emit_bc(hp):
                        x2 = x2_tiles.pop(hp)
                        ps2 = pb.tile([BC, 2, W], F32, tag="b")
                        for q in range(2):
                            nc.tensor.matmul(
                                ps2[:, q, :],
                                lhsT=t_mbct[:],
                                rhs=x2[:, q, :],
                                start=True,
                                stop=True,
                            )
                        hb, hr = divmod(2 * hp, HB)
                        if hr == 0:
                            st = stpool.tile([BC, HB, W], DEV_DT, tag="s")
                            stage[0] = st
                        if hp % 2 == 0:
                            nc.scalar.copy(stage[0][:, hr : hr + 2, :], ps2[:])
                        else:
                            nc.vector.tensor_copy(
                                stage[0][:, hr : hr + 2, :], ps2[:]
                            )
                        if hr == HB - 2:
                            nc.sync.dma_start(
                                out=out_r[:, hb * HB : (hb + 1) * HB, :],
                                in_=stage[0][:],
                            )

                    for hp in range(nhp):
                        emit_w(hp)
                        if hp >= SKEW:
                            emit_bc(hp - SKEW)
                    for hp in range(nhp - SKEW, nhp):
                        emit_bc(hp)
    nc.finalize()
    return nc


def make_host_data(k_size):
    """Blur matrices + per-core slab geometry for sigma=k_size."""
    sigma = float(k_size)
    w, radius = _gauss_kernel(sigma)
    hs = HS
    hin = hs + 2 * radius
    assert hin <= min(H, 128), (hin, H)

    A0 = _blur_matrix(B, w, radius)
    A1 = _blur_matrix(C, w, radius)
    A2 = _blur_matrix(H, w, radius)
    A3 = _blur_matrix(W, w, radius)

    # band-structure sanity: chunk j' of A3^T only reaches cols [lo, hi)
    A3T = A3.T
    for j in range(NJ):
        lo, hi = max(0, 128 * j - radius), min(W, 128 * j + 128 + radius)
        assert (
            np.abs(np.delete(A3T[128 * j : 128 * (j + 1)], np.s_[lo:hi], axis=1)).max()
            == 0.0
        )

    a3t_dev = np.ascontiguousarray(A3T.reshape(NJ, 128, W).astype(DEV_NP))
    mbct_dev = np.ascontiguousarray(np.kron(A0, A1).T.astype(DEV_NP))

    h0s, a2lts = [], []
    for m in range(N_CORES):
        h0 = min(max(hs * m - radius, 0), H - hin)
        rows = A2[hs * m : hs * (m + 1), :]
        mask = np.ones(H, bool)
        mask[h0 : h0 + hin] = False
        assert np.abs(rows[:, mask]).max() == 0.0, m
        h0s.append(h0)
        a2lts.append(np.ascontiguousarray(rows[:, h0 : h0 + hin].T.astype(DEV_NP)))
    return radius, hin, h0s, a2lts, a3t_dev, mbct_dev


def make_in_maps(x_f16, hin, h0s, a2lts, a3t_dev, mbct_dev):
    return [
        {
            "xs": np.ascontiguousarray(x_f16[:, :, h0s[m] : h0s[m] + hin, :]),
            "a2lt": a2lts[m],
            "a3t": a3t_dev,
            "mbct": mbct_dev,
        }
        for m in range(N_CORES)
    ]


_CACHE = {}


def kernel(x, k_size):
    x = np.ascontiguousarray(np.asarray(x, dtype=np.float32))
    assert x.shape == (B, C, H, W), x.shape
    sigma = float(k_size)

    if sigma not in _CACHE:
        radius, hin, h0s, a2lts, a3t_dev, mbct_dev = make_host_data(sigma)
        nc = build_program(hin, radius)
        _CACHE[sigma] = (nc, hin, h0s, a2lts, a3t_dev, mbct_dev)

    nc, hin, h0s, a2lts, a3t_dev, mbct_dev = _CACHE[sigma]
    in_maps = make_in_maps(x.astype(DEV_NP), hin, h0s, a2lts, a3t_dev, mbct_dev)
    res = run_bass_kernel_spmd(nc, in_maps, core_ids=list(range(N_CORES)))
    out = np.concatenate(
        [res.results[m]["out"] for m in range(N_CORES)], axis=2
    )
    return out.astype(np.float32)


# revision 10
# speedup vs baseline: 3.8301x; 3.8301x over previous
"""Trainium2 Bass kernel for nn_BlurTensor: gaussian_filter(sigma=k_size) over
ALL axes of x (B=32, C=3, H=512, W=512) with 'symmetric' boundary.

Decomposition: the blur is the linear operator A0 (x) A1 (x) A2 (x) A3 applied
as mode products (one blur matrix per axis, built on host from k_size).
B and C fold into a single 96x96 Kronecker-product mixing matrix (96 <= 128
partitions), so the device does three matmul passes: H, W (banded), BC.

Sharding: H axis split into 8 x 64-row output slabs; each core receives a
104-row input slab (radius-20 halo), which makes all passes core-local.

Device compute dtype is float16 for all streamed data and blur matrices
(PSUM accumulation stays fp32): halves HBM traffic and weight-load time.
Measured end-to-end max rel err ~7e-4 (tolerance 2e-2).

Device pipeline per core (all intermediates SBUF-resident):
  pass H : out[w, (bc,h)] via lhsT = data slab [h'=104, w-chunk 128],
           rhs = A2_local^T [104, 64]  -> X1 [P:w(4x128), F:(wc,bc,h)]
  pass W : banded 512x512 matrix, 4 chunked matmuls accumulating into one
           PSUM bank per h (start=True on first clears has_written; the
           overlapping band writes then accumulate-or-overwrite per element)
  pass BC: Kronecker matrix as stationary weights, rhs = X2h [96, 512]
"""

import numpy as np

import concourse.bass as bass
import concourse.bacc as bacc
import concourse.mybir as mybir
from concourse.tile import TileContext
from concourse.bass_utils import run_bass_kernel_spmd

TRUNCATE = 4.0
N_CORES = 8
F32 = mybir.dt.float32
DEV_DT = mybir.dt.float16
DEV_NP = np.float16

# problem geometry (hardcoded per contest contract)
B, C, H, W = 32, 3, 512, 512
BC = B * C
HS = H // N_CORES          # 64 output rows per core
NJ = W // 128              # w chunks


def _gauss_kernel(sigma):
    # matches scipy/jax _gaussian_kernel1d in float32, like the reference
    radius = int(TRUNCATE * sigma + 0.5)
    x = np.arange(-radius, radius + 1, dtype=np.float32)
    w = np.exp(-0.5 * (x / sigma) ** 2).astype(np.float32)
    w = (w / w.sum(dtype=np.float32)).astype(np.float32)
    return w.astype(np.float64), radius


def _blur_matrix(L, w, radius):
    """(L, L) float64 operator: blur(v) = A @ v with symmetric padding."""
    I = np.eye(L, dtype=np.float64)
    Ipad = np.pad(I, ((radius, radius), (0, 0)), mode="symmetric")
    A = np.empty((L, L), dtype=np.float64)
    for i in range(L):
        A[i, :] = w @ Ipad[i : i + 2 * radius + 1, :]
    return A


def build_program(hin, radius, reps=1):
    """SPMD Bass program (identical on all cores). `reps` repeats the whole
    blur body inside one NEFF (used by test.py slope timing)."""
    hs = HS
    assert BC <= 128 and hin <= 128 and W % 128 == 0

    nc = bacc.Bacc("TRN2")
    # xs is HOST-PRE-TRANSPOSED to [hin, BC, W]: partition dim (h') first, so
    # every input DMA reads fully contiguous per-partition runs
    xs = nc.dram_tensor("xs", [hin, BC, W], DEV_DT, kind="ExternalInput")
    a2lt = nc.dram_tensor("a2lt", [hin, hs], DEV_DT, kind="ExternalInput")
    a3t = nc.dram_tensor("a3t", [NJ, 128, W], DEV_DT, kind="ExternalInput")
    mbct = nc.dram_tensor("mbct", [BC, BC], DEV_DT, kind="ExternalInput")
    out = nc.dram_tensor("out", [B, C, hs, W], DEV_DT, kind="ExternalOutput")

    xs_r = xs[:]
    out_r = out[:].rearrange("b c h w -> (b c) h w")

    GRP = 8     # bc per matmul/psum group
    LDG = 16    # bc per input DMA (2 matmul groups -> ~1.7MB per transfer)
    n_ld = BC // LDG
    HB = 8      # h rows per output stage/DMA

    def band(j):
        return max(0, 128 * j - radius), min(W, 128 * j + 128 + radius)

    with TileContext(nc) as tc:
        with (
            tc.tile_pool(name="const", bufs=1) as cpool,
            tc.tile_pool(name="x1p", bufs=2) as x1pool,
            tc.tile_pool(name="ld", bufs=3) as ldpool,
            tc.tile_pool(name="psh", bufs=2, space="PSUM") as ph,
            tc.tile_pool(name="x2p", bufs=4) as x2pool,
            tc.tile_pool(name="stg", bufs=2) as stpool,
            tc.tile_pool(name="psw", bufs=2, space="PSUM") as pw,
            tc.tile_pool(name="psb", bufs=2, space="PSUM") as pb,
            # PSUM budget (8 banks): psh 2x[128,1024]=4, psw 2x[96,512]=2,
            # psb 2x[96,512]=2
        ):
            t_a2lt = cpool.tile([hin, hs], DEV_DT)
            nc.sync.dma_start(out=t_a2lt[:], in_=a2lt[:])
            t_a3t = cpool.tile([128, NJ, W], DEV_DT)
            nc.sync.dma_start(out=t_a3t[:], in_=a3t[:].rearrange("j p n -> p j n"))
            t_mbct = cpool.tile([BC, BC], DEV_DT)
            nc.sync.dma_start(out=t_mbct[:], in_=mbct[:])

            for _rep in range(reps):
                # t_x1 ping-pongs across reps so rep r+1's pass H overlaps
                # rep r's pass W/BC
                t_x1 = x1pool.tile([128, NJ, BC, hs], DEV_DT, tag="x1")

                # ---------------- pass H (DMA-streamed groups) ----------------
                for ld in range(n_ld):
                    xt = ldpool.tile([hin, LDG, W], DEV_DT, tag="xt")
                    nc.sync.dma_start(
                        out=xt[:], in_=xs_r[:, ld * LDG : (ld + 1) * LDG, :]
                    )
                    # one 2-bank PSUM tile per (ld, j): 16 bc x 64 h,
                    # copied out in a single [128, 1024] op
                    for j in range(NJ):
                        ps = ph.tile([128, LDG * hs], F32, tag="ph")
                        for i in range(LDG):
                            nc.tensor.matmul(
                                ps[:, i * hs : (i + 1) * hs],
                                lhsT=xt[:, i, 128 * j : 128 * (j + 1)],
                                rhs=t_a2lt[:],
                                start=True,
                                stop=True,
                            )
                        dst = t_x1[:, j, ld * LDG : (ld + 1) * LDG, :]
                        if (ld * NJ + j) % 2 == 0:
                            nc.vector.tensor_copy(dst, ps[:])
                        else:
                            nc.scalar.copy(dst, ps[:])

                # ------------- pass W + BC (fused, skewed per h) --------------
                SKEW = 2
                x2_tiles = {}
                stage = [None]

                def emit_w(h):
                    ps = pw.tile([BC, W], F32, tag="w")
                    for j in range(NJ):
                        lo, hi = band(j)
                        nc.tensor.matmul(
                            ps[:, lo:hi],
                            lhsT=t_x1[:, j, :, h],
                            rhs=t_a3t[:, j, lo:hi],
                            start=(j == 0),
                            stop=(j == NJ - 1),
                        )
                    x2 = x2pool.tile([BC, W], DEV_DT, tag="x2")
                    if h % 2 == 0:
                        nc.vector.tensor_copy(x2[:], ps[:])
                    else:
                        nc.scalar.copy(x2[:], ps[:])
                    x2_tiles[h] = x2

                def emit_bc(h):
                    x2 = x2_tiles.pop(h)
                    ps2 = pb.tile([BC, W], F32, tag="b")
                    nc.tensor.matmul(
                        ps2[:], lhsT=t_mbct[:], rhs=x2[:], start=True, stop=True
                    )
                    hb, hr = divmod(h, HB)
                    if hr == 0:
                        st = stpool.tile([BC, HB, W], DEV_DT, tag="s")
                        stage[0] = st
                    if h % 2 == 0:
                        nc.scalar.copy(stage[0][:, hr, :], ps2[:])
                    else:
                        nc.vector.tensor_copy(stage[0][:, hr, :], ps2[:])
                    if hr == HB - 1:
                        nc.sync.dma_start(
                            out=out_r[:, hb * HB : (hb + 1) * HB, :],
                            in_=stage[0][:],
                        )

                for h in range(hs):
                    emit_w(h)
                    if h >= SKEW:
                        emit_bc(h - SKEW)
                for h in range(hs - SKEW, hs):
                    emit_bc(h)
    nc.finalize()
    return nc


def make_host_data(k_size):
    """Blur matrices + per-core slab geometry for sigma=k_size."""
    sigma = float(k_size)
    w, radius = _gauss_kernel(sigma)
    hs = HS
    hin = hs + 2 * radius
    assert hin <= min(H, 128), (hin, H)

    A0 = _blur_matrix(B, w, radius)
    A1 = _blur_matrix(C, w, radius)
    A2 = _blur_matrix(H, w, radius)
    A3 = _blur_matrix(W, w, radius)

    # band-structure sanity: chunk j' of A3^T only reaches cols [lo, hi)
    A3T = A3.T
    for j in range(NJ):
        lo, hi = max(0, 128 * j - radius), min(W, 128 * j + 128 + radius)
        assert (
            np.abs(np.delete(A3T[128 * j : 128 * (j + 1)], np.s_[lo:hi], axis=1)).max()
            == 0.0
        )

    a3t_dev = np.ascontiguousarray(A3T.reshape(NJ, 128, W).astype(DEV_NP))
    mbct_dev = np.ascontiguousarray(np.kron(A0, A1).T.astype(DEV_NP))

    h0s, a2lts = [], []
    for m in range(N_CORES):
        h0 = min(max(hs * m - radius, 0), H - hin)
        rows = A2[hs * m : hs * (m + 1), :]
        mask = np.ones(H, bool)
        mask[h0 : h0 + hin] = False
        assert np.abs(rows[:, mask]).max() == 0.0, m
        h0s.append(h0)
        a2lts.append(np.ascontiguousarray(rows[:, h0 : h0 + hin].T.astype(DEV_NP)))
    return radius, hin, h0s, a2lts, a3t_dev, mbct_dev


def make_in_maps(x_f16, hin, h0s, a2lts, a3t_dev, mbct_dev):
    # xs shipped as [hin, BC, W] (see build_program): contiguous per-partition
    return [
        {
            "xs": np.ascontiguousarray(
                x_f16[:, :, h0s[m] : h0s[m] + hin, :]
                .reshape(BC, hin, W)
                .transpose(1, 0, 2)
            ),
            "a2lt": a2lts[m],
            "a3t": a3t_dev,
            "mbct": mbct_dev,
        }
        for m in range(N_CORES)
    ]


_CACHE = {}


def kernel(x, k_size):
    x = np.ascontiguousarray(np.asarray(x, dtype=np.float32))
    assert x.shape == (B, C, H, W), x.shape
    sigma = float(k_size)

    if sigma not in _CACHE:
        radius, hin, h0s, a2lts, a3t_dev, mbct_dev = make_host_data(sigma)
        nc = build_program(hin, radius)
        _CACHE[sigma] = (nc, hin, h0s, a2lts, a3t_dev, mbct_dev)

    nc, hin, h0s, a2lts, a3t_dev, mbct_dev = _CACHE[sigma]
    in_maps = make_in_maps(x.astype(DEV_NP), hin, h0s, a2lts, a3t_dev, mbct_dev)
    res = run_bass_kernel_spmd(nc, in_maps, core_ids=list(range(N_CORES)))
    out = np.concatenate(
        [res.results[m]["out"] for m in range(N_CORES)], axis=2
    )
    return out.astype(np.float32)
